# revision 9
# baseline (speedup 1.0000x reference)
"""Trainium2 Bass kernel for nn_CascadeTransformerMM (4-layer ternary-GLU cascade).

Math (per layer, per token row):
  h   = rms_scale * x * rsqrt(mean(x^2) + 1e-6)
  s   = clip(127/(max|h| + 1e-5), 1e-3, 1e3);  q = round(s*h)      (ints in [-127,127])
  Wt  = clip(round(W * 127/(max|W| + 1e-5)), -1, 1)                 (ternary {-1,0,1})
  u   = (q @ Wg_t)/s ; v = (q @ Wu_t)/s ; g = silu(u)*v
  s2  = clip(127/(max|g| + 1e-5), 1e-3, 1e3); gq = round(s2*g)
  x  += (gq @ Wd_t)/s2

Distribution: pure data-parallel over the batch dim (8 batches -> 8 cores),
weights replicated per core. Per-matrix |W|max is computed cooperatively:
each core reduces a 1/8 row-slice, then a tiny AllReduce(max) shares the 12
scalars. All matmuls run on the PE array with bf16 activations (integers
<= 127, exact) x fp8 ternary weights (exact), fp32 PSUM accumulation -> the
heavy compute is bit-exact integer arithmetic.

Layout: token-major activations [128 tok, D]; q/gq transposed to feature-major
via single xbar DMA-transposes; weights ternarized on the fly into SBUF-resident
fp8 tiles (int32-convert round + chained min/max clip, both exact)."""

import os
import sys

for _p in ("/opt/trn_rl_repo", "/root/.axon_site/_ro/trn_rl_repo"):
    if os.path.isdir(_p) and _p not in sys.path:
        sys.path.insert(0, _p)

import numpy as np
from contextlib import ExitStack

import concourse.bass as bass
import concourse.mybir as mybir
import concourse.tile as tile
from concourse.bass_utils import run_bass_kernel_spmd

dt = mybir.dt
AF = mybir.ActivationFunctionType
ALU = mybir.AluOpType

MAGIC = float(1.5 * 2**23)  # fp32 round-to-nearest-even magic constant
D = 1024
F = 4096
L = 4
NCORES = 8
TOK = 1024  # tokens per core (one batch of S=1024)

NDK = D // 128   # 8 contraction tiles for up-proj
NFT = F // 128   # 32 contraction tiles for down-proj
NFC = F // 512   # 8 free-dim chunks for up-proj
NTT = TOK // 128  # 8 token tiles


def _split_excess_waits(nc, max_waits: int = 1) -> int:
    """walrus in this container rejects >1 sync-wait per instruction; split
    extras into standalone event-semaphore waits on the same engine (same-
    engine program order makes this semantically identical)."""
    n = 0
    for func in nc.m.functions:
        for block in func.blocks:
            changed = False
            out = []
            for inst in block.instructions:
                si = getattr(inst, "sync_info", None)
                if si is not None and si.on_wait and len(si.on_wait) > max_waits:
                    waits = list(si.on_wait)
                    for j, w in enumerate(waits[max_waits:]):
                        out.append(
                            mybir.InstEventSemaphore(
                                name=f"{inst.name}-xw{j}",
                                engine=inst.engine,
                                ins=[],
                                outs=[],
                                sync_info=mybir.SyncInfo(on_wait=[w], on_update=[]),
                            )
                        )
                        n += 1
                    inst.sync_info = mybir.SyncInfo(
                        on_wait=waits[:max_waits], on_update=list(si.on_update)
                    )
                    changed = True
                out.append(inst)
            if changed:
                block.instructions = out
    return n


def build(n_cores: int = NCORES, n_tok_tiles: int = NTT, n_layers: int = L) -> bass.Bass:
    nc = bass.Bass(num_devices=n_cores)
    tok = n_tok_tiles * 128

    x_ext = nc.declare_dram_parameter("x", [tok, D], dt.float32, isOutput=False)
    rs_ext = nc.declare_dram_parameter("rs", [n_layers, D], dt.float32, isOutput=False)
    wg_ext = nc.declare_dram_parameter("wg", [n_layers, D, F], dt.float32, isOutput=False)
    wu_ext = nc.declare_dram_parameter("wu", [n_layers, D, F], dt.float32, isOutput=False)
    wd_ext = nc.declare_dram_parameter("wd", [n_layers, F, D], dt.float32, isOutput=False)
    # per-core row-slices of each matrix for the cooperative |W|max
    gsl = D // n_cores
    dsl = F // n_cores
    slg_ext = nc.declare_dram_parameter("slg", [n_layers, gsl, F], dt.float32, isOutput=False)
    slu_ext = nc.declare_dram_parameter("slu", [n_layers, gsl, F], dt.float32, isOutput=False)
    sld_ext = nc.declare_dram_parameter("sld", [n_layers, dsl, D], dt.float32, isOutput=False)
    out_ext = nc.declare_dram_parameter("out", [tok, D], dt.float32, isOutput=True)

    nmat = 3 * n_layers
    mx_loc = nc.dram_tensor("mx_loc", [1, 16], dt.float32)
    mx_glob = nc.dram_tensor("mx_glob", [1, 16], dt.float32)

    with tile.TileContext(nc) as tc, ExitStack() as ctx:
        P = ctx.enter_context
        wpool = P(tc.tile_pool(name="wts", bufs=3))
        wstream = P(tc.tile_pool(name="wstream", bufs=2))
        wi32 = P(tc.tile_pool(name="wi32", bufs=2))
        xpool = P(tc.tile_pool(name="x1", bufs=2))
        s4k = P(tc.tile_pool(name="s4k", bufs=2))       # t1_tmp / xnew scratch
        qpool = P(tc.tile_pool(name="q", bufs=1))
        qtpool = P(tc.tile_pool(name="qt", bufs=2))
        silupool = P(tc.tile_pool(name="silu", bufs=1))
        gpool = P(tc.tile_pool(name="g", bufs=2))
        gqpool = P(tc.tile_pool(name="gq", bufs=1))
        gqtpool = P(tc.tile_pool(name="gqt", bufs=2))
        bcpool = P(tc.tile_pool(name="bc", bufs=1))
        batch = P(tc.tile_pool(name="batch", bufs=2))   # [128, ntt] per-layer stats
        sc = P(tc.tile_pool(name="sc", bufs=4))         # [128, small] scalars
        xdrpool = P(tc.tile_pool(name="xdr", bufs=2))
        const = P(tc.tile_pool(name="const", bufs=1))
        dram = P(tc.tile_pool(name="dram", bufs=2, space="DRAM"))
        psA = P(tc.tile_pool(name="psA", bufs=2, space="PSUM"))
        psB = P(tc.tile_pool(name="psB", bufs=2, space="PSUM"))
        psD = P(tc.tile_pool(name="psD", bufs=2, space="PSUM"))
        psM = P(tc.tile_pool(name="psM", bufs=1, space="PSUM"))

        # ---------- constants ----------
        ones1 = const.tile([1, 128], dt.float32, tag="ones1")
        nc.gpsimd.memset(ones1[:], 1.0)
        mag = const.tile([128, 1], dt.float32, tag="mag")
        nc.gpsimd.memset(mag[:], MAGIC)
        nmag = const.tile([128, 1], dt.float32, tag="nmag")
        nc.gpsimd.memset(nmag[:], -MAGIC)
        c127 = const.tile([128, 16], dt.float32, tag="c127")
        nc.gpsimd.memset(c127[:], 127.0)

        # ---------- cooperative per-matrix |W|max ----------
        wmax_cols = const.tile([128, 16], dt.float32, tag="wmaxc")
        nc.gpsimd.memset(wmax_cols[:], 0.0)
        for l in range(n_layers):
            for mi, (ext, rows, cols) in enumerate(
                ((slg_ext, gsl, F), (slu_ext, gsl, F), (sld_ext, dsl, D))
            ):
                idx = 3 * l + mi
                nslab = rows // 128
                nch = cols // 1024
                nunit = nslab * nch
                part = sc.tile([128, nunit], dt.float32, tag="wmaxpart")
                for sl in range(nslab):
                    for ch in range(nch):
                        wt = wstream.tile([128, 1024], dt.float32, tag="wstream")
                        nc.sync.dma_start(
                            wt[:],
                            ext[l, sl * 128:(sl + 1) * 128, ch * 1024:(ch + 1) * 1024],
                        )
                        nc.vector.tensor_reduce(
                            part[:, sl * nch + ch: sl * nch + ch + 1],
                            wt[:],
                            axis=mybir.AxisListType.X,
                            op=ALU.max,
                            apply_absolute_value=True,
                        )
                nc.vector.tensor_reduce(
                    wmax_cols[:, idx:idx + 1],
                    part[:, 0:nunit],
                    axis=mybir.AxisListType.X,
                    op=ALU.max,
                    apply_absolute_value=False,
                )
        # partition-dim reduce 128 -> 1 (gpsimd), pad row to 16, AllReduce(max)
        mrow = const.tile([1, 16], dt.float32, tag="mrow")
        nc.gpsimd.memset(mrow[:], 0.0)
        nc.gpsimd.tensor_reduce(
            mrow[:, 0:nmat], wmax_cols[:, 0:nmat], axis=mybir.AxisListType.C, op=ALU.max
        )
        nc.sync.dma_start(mx_loc[:], mrow[:])
        nc.gpsimd.collective_compute(
            "AllReduce",
            ALU.max,
            replica_groups=[list(range(n_cores))],
            ins=[mx_loc[:].opt()],
            outs=[mx_glob[:].opt()],
        )
        grow = const.tile([1, 16], dt.float32, tag="grow")
        nc.sync.dma_start(grow[:], mx_glob[:])
        # broadcast to all partitions via PE outer product; wsc = 127/(m+1e-5)
        msc_ps = psM.tile([128, D], dt.float32, tag="bcps")
        nc.tensor.matmul(msc_ps[:, 0:16], ones1[:], grow[:], start=True, stop=True)
        msc = const.tile([128, 16], dt.float32, tag="msc")
        nc.vector.tensor_scalar(msc[:], msc_ps[:, 0:16], 1e-5, None, op0=ALU.add)
        mrec = const.tile([128, 16], dt.float32, tag="mrec")
        nc.vector.reciprocal(mrec[:], msc[:])
        wsc = const.tile([128, 16], dt.float32, tag="wsc")
        nc.vector.tensor_scalar(wsc[:], mrec[:], 127.0, None, op0=ALU.mult)

        # ---------- layers ----------
        xsrc = x_ext
        for l in range(n_layers):
            # ---- ternarize weights into SBUF-resident fp8 tiles ----
            wg_t = wpool.tile([128, NDK, F], dt.float8e4, tag="wts")
            wu_t = wpool.tile([128, NDK, F], dt.float8e4, tag="wts")
            wd_t = wpool.tile([128, NFT, D], dt.float8e4, tag="wts")
            for ch in range(F // 1024):  # chunk-outer: early F-chunks ready first
                for dk in range(NDK):
                    for ext, wt_dst, idx in ((wg_ext, wg_t, 3 * l), (wu_ext, wu_t, 3 * l + 1)):
                        wtile = wstream.tile([128, 1024], dt.float32, tag="wstream")
                        nc.sync.dma_start(
                            wtile[:],
                            ext[l, dk * 128:(dk + 1) * 128, ch * 1024:(ch + 1) * 1024],
                        )
                        r32 = wi32.tile([128, 1024], dt.int32, tag="wi32")
                        nc.vector.tensor_scalar(
                            r32[:], wtile[:], wsc[:, idx:idx + 1], None, op0=ALU.mult
                        )
                        nc.vector.tensor_scalar(
                            wt_dst[:, dk, ch * 1024:(ch + 1) * 1024],
                            r32[:], 1, -1, op0=ALU.min, op1=ALU.max,
                        )
            for ft in range(NFT):
                wtile = wstream.tile([128, 1024], dt.float32, tag="wstream")
                nc.sync.dma_start(wtile[:], wd_ext[l, ft * 128:(ft + 1) * 128, :])
                r32 = wi32.tile([128, 1024], dt.int32, tag="wi32")
                nc.gpsimd.tensor_scalar(
                    r32[:], wtile[:], wsc[:, 3 * l + 2:3 * l + 3], None, op0=ALU.mult
                )
                nc.gpsimd.tensor_scalar(
                    wd_t[:, ft, :], r32[:], 1, -1, op0=ALU.min, op1=ALU.max
                )

            # ---- rms_scale broadcast [128, D] ----
            rs_row = const.tile([1, D], dt.float32, tag="rsrow")
            nc.sync.dma_start(rs_row[:], rs_ext[l:l + 1, :])
            bc_ps = psM.tile([128, D], dt.float32, tag="bcps")
            for h in range(D // 512):
                nc.tensor.matmul(
                    bc_ps[:, h * 512:(h + 1) * 512], ones1[:],
                    rs_row[:, h * 512:(h + 1) * 512], start=True, stop=True,
                )
            scale_bc = bcpool.tile([128, D], dt.float32, tag="bc")
            nc.scalar.activation(scale_bc[:], bc_ps[:], AF.Copy)

            # ---- phase A: per-token-tile row stats (ssq, max|h_pre|) ----
            ssq_all = batch.tile([128, n_tok_tiles], dt.float32, tag="ssq")
            mx_all = batch.tile([128, n_tok_tiles], dt.float32, tag="mx")
            for i in range(n_tok_tiles):
                x1 = xpool.tile([128, D], dt.float32, tag="x1")
                nc.gpsimd.dma_start(x1[:], xsrc[i * 128:(i + 1) * 128, :])
                tt = s4k.tile([128, D], dt.float32, tag="s4k")
                nc.scalar.activation(
                    tt[:], x1[:], AF.Square, accum_out=ssq_all[:, i:i + 1]
                )
                nc.vector.tensor_tensor(tt[:], x1[:], scale_bc[:], op=ALU.mult)
                nc.vector.tensor_reduce(
                    mx_all[:, i:i + 1], tt[:], axis=mybir.AxisListType.X,
                    op=ALU.max, apply_absolute_value=True,
                )
            # batched stats: rstd = 1/sqrt(mean+eps); s = clip(127/(rstd*mx+1e-5))
            ms = batch.tile([128, n_tok_tiles], dt.float32, tag="ms")
            nc.vector.tensor_scalar(ms[:], ssq_all[:], 1.0 / D, 1e-6, op0=ALU.mult, op1=ALU.add)
            rt = batch.tile([128, n_tok_tiles], dt.float32, tag="rt")
            nc.scalar.activation(rt[:], ms[:], AF.Sqrt)
            rstd = batch.tile([128, n_tok_tiles], dt.float32, tag="rstd")
            nc.vector.reciprocal(rstd[:], rt[:])
            # one Newton step: rstd *= 1.5 - 0.5*ms*rstd^2  (fixes the ~7e-6
            # Sqrt-LUT error that quantization tie-flips amplify layer by layer)
            nwt = batch.tile([128, n_tok_tiles], dt.float32, tag="nwt")
            nc.vector.tensor_tensor(nwt[:], rstd[:], rstd[:], op=ALU.mult)
            nc.vector.tensor_tensor(nwt[:], nwt[:], ms[:], op=ALU.mult)
            nc.vector.tensor_scalar(nwt[:], nwt[:], -0.5, 1.5, op0=ALU.mult, op1=ALU.add)
            nc.vector.tensor_tensor(rstd[:], rstd[:], nwt[:], op=ALU.mult)
            maxh = batch.tile([128, n_tok_tiles], dt.float32, tag="maxh")
            nc.vector.tensor_tensor(maxh[:], mx_all[:], rstd[:], op=ALU.mult)
            nc.vector.tensor_scalar(maxh[:], maxh[:], 1e-5, None, op0=ALU.add)
            sr = batch.tile([128, n_tok_tiles], dt.float32, tag="sr")
            nc.vector.reciprocal(sr[:], maxh[:])
            s_all = batch.tile([128, n_tok_tiles], dt.float32, tag="s_all")
            nc.vector.tensor_scalar(s_all[:], sr[:], 127.0, 1e3, op0=ALU.mult, op1=ALU.min)
            nc.vector.tensor_scalar(s_all[:], s_all[:], 1e-3, None, op0=ALU.max)
            c1_all = batch.tile([128, n_tok_tiles], dt.float32, tag="c1")
            nc.vector.tensor_tensor(c1_all[:], s_all[:], rstd[:], op=ALU.mult)
            rs_all = batch.tile([128, n_tok_tiles], dt.float32, tag="rs_all")
            nc.vector.reciprocal(rs_all[:], s_all[:])

            # ---- phase B: per token tile, full GLU ----
            xdst = out_ext if l == n_layers - 1 else dram.tile([tok, D], dt.float32, tag="xbuf")
            for i in range(n_tok_tiles):
                x1 = xpool.tile([128, D], dt.float32, tag="x1")
                nc.gpsimd.dma_start(x1[:], xsrc[i * 128:(i + 1) * 128, :])
                g = gpool.tile([128, F], dt.float32, tag="g")
                t1 = g[:, 0:D]  # scratch alias: consumed before g chunks are written
                nc.vector.tensor_tensor(t1[:], x1[:], scale_bc[:], op=ALU.mult)
                # q = round(c1 * t1) via magic add/sub (exact RNE), out bf16
                nc.scalar.activation(t1[:], t1[:], AF.Identity,
                                     scale=c1_all[:, i:i + 1], bias=mag[:])
                q = qpool.tile([128, D], dt.bfloat16, tag="q")
                nc.scalar.activation(q[:], t1[:], AF.Identity, bias=nmag[:])
                qT = qtpool.tile([128, NDK, 128], dt.bfloat16, tag="qt")
                nc.sync.dma_start_transpose(qT[:], q[:])

                gm8 = sc.tile([128, NFC], dt.float32, tag="gm8")
                for f in range(NFC):
                    u_ps = psA.tile([128, 512], dt.float32, tag="ups")
                    v_ps = psB.tile([128, 512], dt.float32, tag="vps")
                    for dk in range(NDK):
                        nc.tensor.matmul(
                            u_ps[:], qT[:, dk, :], wg_t[:, dk, f * 512:(f + 1) * 512],
                            start=(dk == 0), stop=(dk == NDK - 1),
                        )
                        nc.tensor.matmul(
                            v_ps[:], qT[:, dk, :], wu_t[:, dk, f * 512:(f + 1) * 512],
                            start=(dk == 0), stop=(dk == NDK - 1),
                        )
                    su = silupool.tile([128, 512], dt.float32, tag="silu")
                    nc.scalar.activation(su[:], u_ps[:], AF.Silu, scale=rs_all[:, i:i + 1])
                    nc.vector.tensor_tensor(
                        g[:, f * 512:(f + 1) * 512], su[:], v_ps[:], op=ALU.mult
                    )
                    nc.vector.tensor_reduce(
                        gm8[:, f:f + 1], g[:, f * 512:(f + 1) * 512],
                        axis=mybir.AxisListType.X, op=ALU.max, apply_absolute_value=True,
                    )
                # s2 = clip(127/(max|g|/s + 1e-5)); c2 = s2/s ; rs2 = 1/s2
                gmx = sc.tile([128, 1], dt.float32, tag="gmx")
                nc.vector.tensor_reduce(
                    gmx[:], gm8[:], axis=mybir.AxisListType.X, op=ALU.max,
                    apply_absolute_value=False,
                )
                nc.vector.tensor_tensor(gmx[:], gmx[:], rs_all[:, i:i + 1], op=ALU.mult)
                nc.vector.tensor_scalar(gmx[:], gmx[:], 1e-5, None, op0=ALU.add)
                s2r = sc.tile([128, 1], dt.float32, tag="s2r")
                nc.vector.reciprocal(s2r[:], gmx[:])
                s2 = sc.tile([128, 1], dt.float32, tag="s2")
                nc.vector.tensor_scalar(s2[:], s2r[:], 127.0, 1e3, op0=ALU.mult, op1=ALU.min)
                nc.vector.tensor_scalar(s2[:], s2[:], 1e-3, None, op0=ALU.max)
                c2 = sc.tile([128, 1], dt.float32, tag="c2")
                nc.vector.tensor_tensor(c2[:], s2[:], rs_all[:, i:i + 1], op=ALU.mult)
                rs2 = sc.tile([128, 1], dt.float32, tag="rs2")
                nc.vector.reciprocal(rs2[:], s2[:])
                # gq = round(c2*g) via magic, out bf16
                nc.scalar.activation(g[:], g[:], AF.Identity, scale=c2[:], bias=mag[:])
                gq = gqpool.tile([128, F], dt.bfloat16, tag="gq")
                nc.scalar.activation(gq[:], g[:], AF.Identity, bias=nmag[:])
                gqT = gqtpool.tile([128, NFT, 128], dt.bfloat16, tag="gqt")
                nc.sync.dma_start_transpose(gqT[:], gq[:])

                xnew = s4k.tile([128, D], dt.float32, tag="s4k")
                for dc in range(D // 512):
                    xd_ps = psD.tile([128, 512], dt.float32, tag="xdps")
                    for ft in range(NFT):
                        nc.tensor.matmul(
                            xd_ps[:], gqT[:, ft, :], wd_t[:, ft, dc * 512:(dc + 1) * 512],
                            start=(ft == 0), stop=(ft == NFT - 1),
                        )
                    xdr = xdrpool.tile([128, 512], dt.float32, tag="xdr")
                    nc.scalar.activation(xdr[:], xd_ps[:], AF.Copy, scale=rs2[:])
                    nc.vector.tensor_tensor(
                        xnew[:, dc * 512:(dc + 1) * 512],
                        x1[:, dc * 512:(dc + 1) * 512], xdr[:], op=ALU.add,
                    )
                nc.sync.dma_start(xdst[i * 128:(i + 1) * 128, :], xnew[:])
            xsrc = xdst

    _split_excess_waits(nc)
    return nc


_nc_cache = {}


def _get_nc(key=(NCORES, NTT, L)):
    if key not in _nc_cache:
        _nc_cache[key] = build(*key)
    return _nc_cache[key]


def _make_in_maps(x, rs, wg, wu, wd, n_cores=NCORES):
    gsl = D // n_cores
    dsl = F // n_cores
    in_maps = []
    for c in range(n_cores):
        in_maps.append({
            "x": x[c],
            "rs": rs,
            "wg": wg,
            "wu": wu,
            "wd": wd,
            "slg": np.ascontiguousarray(wg[:, c * gsl:(c + 1) * gsl, :]),
            "slu": np.ascontiguousarray(wu[:, c * gsl:(c + 1) * gsl, :]),
            "sld": np.ascontiguousarray(wd[:, c * dsl:(c + 1) * dsl, :]),
        })
    return in_maps


def kernel(x, rms_scale, W_g, W_u, W_d):
    """Full-input entry point: shard over batch, run 8-core SPMD, gather."""
    x = np.ascontiguousarray(np.asarray(x, dtype=np.float32))
    rs = np.ascontiguousarray(np.asarray(rms_scale, dtype=np.float32))
    wg = np.ascontiguousarray(np.asarray(W_g, dtype=np.float32))
    wu = np.ascontiguousarray(np.asarray(W_u, dtype=np.float32))
    wd = np.ascontiguousarray(np.asarray(W_d, dtype=np.float32))
    B, S, Dx = x.shape
    assert (B, S, Dx) == (NCORES, TOK, D), (B, S, Dx)
    nc = _get_nc()
    in_maps = _make_in_maps(x, rs, wg, wu, wd)
    res = run_bass_kernel_spmd(nc, in_maps, list(range(NCORES)))
    return np.stack([res.results[c]["out"] for c in range(NCORES)], axis=0)


# revision 10
# speedup vs baseline: 1.5960x; 1.5960x over previous
"""Trainium2 Bass kernel for nn_CascadeTransformerMM (4-layer ternary-GLU cascade).

Math (per layer, per token row):
  h   = rms_scale * x * rsqrt(mean(x^2) + 1e-6)
  s   = clip(127/(max|h| + 1e-5), 1e-3, 1e3);  q = round(s*h)      (ints in [-127,127])
  Wt  = clip(round(W * 127/(max|W| + 1e-5)), -1, 1)                 (ternary {-1,0,1})
  u   = (q @ Wg_t)/s ; v = (q @ Wu_t)/s ; g = silu(u)*v
  s2  = clip(127/(max|g| + 1e-5), 1e-3, 1e3); gq = round(s2*g)
  x  += (gq @ Wd_t)/s2

Distribution: pure data-parallel over the batch dim (8 batches -> 8 cores),
weights replicated per core. Per-matrix |W|max is computed cooperatively:
each core reduces a 1/8 row-slice, then a tiny AllReduce(max) shares the 12
scalars. All matmuls run on the PE array with bf16 activations (integers
<= 127, exact) x fp8 ternary weights (exact), fp32 PSUM accumulation -> the
heavy compute is bit-exact integer arithmetic.

Layout: token-major activations [128 tok, D]; q/gq transposed to feature-major
via single xbar DMA-transposes; weights ternarized on the fly into SBUF-resident
fp8 tiles (int32-convert round + chained min/max clip, both exact)."""

import os
import sys

for _p in ("/opt/trn_rl_repo", "/root/.axon_site/_ro/trn_rl_repo"):
    if os.path.isdir(_p) and _p not in sys.path:
        sys.path.insert(0, _p)

import numpy as np
from contextlib import ExitStack

import concourse.bass as bass
import concourse.mybir as mybir
import concourse.tile as tile
from concourse.bass_utils import run_bass_kernel_spmd

dt = mybir.dt
AF = mybir.ActivationFunctionType
ALU = mybir.AluOpType

MAGIC = float(1.5 * 2**23)  # fp32 round-to-nearest-even magic constant
D = 1024
F = 4096
L = 4
NCORES = 8
TOK = 1024  # tokens per core (one batch of S=1024)

NDK = D // 128   # 8 contraction tiles for up-proj
NFT = F // 128   # 32 contraction tiles for down-proj
NFC = F // 512   # 8 free-dim chunks for up-proj
NTT = TOK // 128  # 8 token tiles


def _split_excess_waits(nc, max_waits: int = 1) -> int:
    """walrus in this container rejects >1 sync-wait per instruction; split
    extras into standalone event-semaphore waits on the same engine (same-
    engine program order makes this semantically identical)."""
    n = 0
    for func in nc.m.functions:
        for block in func.blocks:
            changed = False
            out = []
            for inst in block.instructions:
                si = getattr(inst, "sync_info", None)
                if si is not None and si.on_wait and len(si.on_wait) > max_waits:
                    waits = list(si.on_wait)
                    for j, w in enumerate(waits[max_waits:]):
                        out.append(
                            mybir.InstEventSemaphore(
                                name=f"{inst.name}-xw{j}",
                                engine=inst.engine,
                                ins=[],
                                outs=[],
                                sync_info=mybir.SyncInfo(on_wait=[w], on_update=[]),
                            )
                        )
                        n += 1
                    inst.sync_info = mybir.SyncInfo(
                        on_wait=waits[:max_waits], on_update=list(si.on_update)
                    )
                    changed = True
                out.append(inst)
            if changed:
                block.instructions = out
    return n


def build(n_cores: int = NCORES, n_tok_tiles: int = NTT, n_layers: int = L) -> bass.Bass:
    nc = bass.Bass(num_devices=n_cores)
    tok = n_tok_tiles * 128

    x_ext = nc.declare_dram_parameter("x", [tok, D], dt.float32, isOutput=False)
    rs_ext = nc.declare_dram_parameter("rs", [n_layers, D], dt.float32, isOutput=False)
    wg_ext = nc.declare_dram_parameter("wg", [n_layers, D, F], dt.float32, isOutput=False)
    wu_ext = nc.declare_dram_parameter("wu", [n_layers, D, F], dt.float32, isOutput=False)
    wd_ext = nc.declare_dram_parameter("wd", [n_layers, F, D], dt.float32, isOutput=False)
    # per-core row-slices of each matrix for the cooperative |W|max
    gsl = D // n_cores
    dsl = F // n_cores
    slg_ext = nc.declare_dram_parameter("slg", [n_layers, gsl, F], dt.float32, isOutput=False)
    slu_ext = nc.declare_dram_parameter("slu", [n_layers, gsl, F], dt.float32, isOutput=False)
    sld_ext = nc.declare_dram_parameter("sld", [n_layers, dsl, D], dt.float32, isOutput=False)
    out_ext = nc.declare_dram_parameter("out", [tok, D], dt.float32, isOutput=True)

    nmat = 3 * n_layers
    mx_loc = nc.dram_tensor("mx_loc", [1, 16], dt.float32)
    mx_glob = nc.dram_tensor("mx_glob", [1, 16], dt.float32)

    with tile.TileContext(nc) as tc, ExitStack() as ctx:
        P = ctx.enter_context
        wpool = P(tc.tile_pool(name="wts", bufs=3))
        wstream = P(tc.tile_pool(name="wstream", bufs=2))
        wi32 = P(tc.tile_pool(name="wi32", bufs=2))
        xpool = P(tc.tile_pool(name="x1", bufs=2))
        s4k = P(tc.tile_pool(name="s4k", bufs=2))       # t1_tmp / xnew scratch
        qpool = P(tc.tile_pool(name="q", bufs=1))
        qtpool = P(tc.tile_pool(name="qt", bufs=2))
        silupool = P(tc.tile_pool(name="silu", bufs=1))
        gpool = P(tc.tile_pool(name="g", bufs=2))
        gqpool = P(tc.tile_pool(name="gq", bufs=1))
        gqtpool = P(tc.tile_pool(name="gqt", bufs=2))
        bcpool = P(tc.tile_pool(name="bc", bufs=1))
        batch = P(tc.tile_pool(name="batch", bufs=2))   # [128, ntt] per-layer stats
        sc = P(tc.tile_pool(name="sc", bufs=4))         # [128, small] scalars
        xdrpool = P(tc.tile_pool(name="xdr", bufs=2))
        const = P(tc.tile_pool(name="const", bufs=1))
        dram = P(tc.tile_pool(name="dram", bufs=2, space="DRAM"))
        psA = P(tc.tile_pool(name="psA", bufs=2, space="PSUM"))
        psB = P(tc.tile_pool(name="psB", bufs=2, space="PSUM"))
        psD = P(tc.tile_pool(name="psD", bufs=2, space="PSUM"))
        psM = P(tc.tile_pool(name="psM", bufs=1, space="PSUM"))

        # ---------- constants ----------
        ones1 = const.tile([1, 128], dt.float32, tag="ones1")
        nc.gpsimd.memset(ones1[:], 1.0)
        mag = const.tile([128, 1], dt.float32, tag="mag")
        nc.gpsimd.memset(mag[:], MAGIC)
        nmag = const.tile([128, 1], dt.float32, tag="nmag")
        nc.gpsimd.memset(nmag[:], -MAGIC)
        c127 = const.tile([128, 16], dt.float32, tag="c127")
        nc.gpsimd.memset(c127[:], 127.0)

        # ---------- cooperative per-matrix |W|max ----------
        wmax_cols = const.tile([128, 16], dt.float32, tag="wmaxc")
        nc.gpsimd.memset(wmax_cols[:], 0.0)
        for l in range(n_layers):
            for mi, (ext, rows, cols) in enumerate(
                ((slg_ext, gsl, F), (slu_ext, gsl, F), (sld_ext, dsl, D))
            ):
                idx = 3 * l + mi
                nslab = rows // 128
                nch = cols // 1024
                nunit = nslab * nch
                part = sc.tile([128, nunit], dt.float32, tag="wmaxpart")
                for sl in range(nslab):
                    for ch in range(nch):
                        wt = wstream.tile([128, 1024], dt.float32, tag="wstream")
                        nc.sync.dma_start(
                            wt[:],
                            ext[l, sl * 128:(sl + 1) * 128, ch * 1024:(ch + 1) * 1024],
                        )
                        nc.vector.tensor_reduce(
                            part[:, sl * nch + ch: sl * nch + ch + 1],
                            wt[:],
                            axis=mybir.AxisListType.X,
                            op=ALU.max,
                            apply_absolute_value=True,
                        )
                nc.vector.tensor_reduce(
                    wmax_cols[:, idx:idx + 1],
                    part[:, 0:nunit],
                    axis=mybir.AxisListType.X,
                    op=ALU.max,
                    apply_absolute_value=False,
                )
        # partition-dim reduce 128 -> 1 (gpsimd), pad row to 16, AllReduce(max)
        mrow = const.tile([1, 16], dt.float32, tag="mrow")
        nc.gpsimd.memset(mrow[:], 0.0)
        nc.gpsimd.tensor_reduce(
            mrow[:, 0:nmat], wmax_cols[:, 0:nmat], axis=mybir.AxisListType.C, op=ALU.max
        )
        nc.sync.dma_start(mx_loc[:], mrow[:])
        nc.gpsimd.collective_compute(
            "AllReduce",
            ALU.max,
            replica_groups=[list(range(n_cores))],
            ins=[mx_loc[:].opt()],
            outs=[mx_glob[:].opt()],
        )
        grow = const.tile([1, 16], dt.float32, tag="grow")
        nc.sync.dma_start(grow[:], mx_glob[:])
        # broadcast to all partitions via PE outer product; wsc = 127/(m+1e-5)
        msc_ps = psM.tile([128, D], dt.float32, tag="bcps")
        nc.tensor.matmul(msc_ps[:, 0:16], ones1[:], grow[:], start=True, stop=True)
        msc = const.tile([128, 16], dt.float32, tag="msc")
        nc.vector.tensor_scalar(msc[:], msc_ps[:, 0:16], 1e-5, None, op0=ALU.add)
        mrec = const.tile([128, 16], dt.float32, tag="mrec")
        nc.vector.reciprocal(mrec[:], msc[:])
        wsc = const.tile([128, 16], dt.float32, tag="wsc")
        nc.vector.tensor_scalar(wsc[:], mrec[:], 127.0, None, op0=ALU.mult)

        # ---------- layers ----------
        xsrc = x_ext
        for l in range(n_layers):
            # ---- ternarize weights into SBUF-resident fp8 tiles ----
            wg_t = wpool.tile([128, NDK, F], dt.float8e4, tag="wts")
            wu_t = wpool.tile([128, NDK, F], dt.float8e4, tag="wts")
            wd_t = wpool.tile([128, NFT, D], dt.float8e4, tag="wts")
            for ch in range(F // 1024):  # chunk-outer: early F-chunks ready first
                for dk in range(NDK):
                    for ext, wt_dst, idx in ((wg_ext, wg_t, 3 * l), (wu_ext, wu_t, 3 * l + 1)):
                        wtile = wstream.tile([128, 1024], dt.float32, tag="wstream")
                        nc.sync.dma_start(
                            wtile[:],
                            ext[l, dk * 128:(dk + 1) * 128, ch * 1024:(ch + 1) * 1024],
                        )
                        r32 = wi32.tile([128, 1024], dt.int32, tag="wi32")
                        nc.vector.tensor_scalar(
                            r32[:], wtile[:], wsc[:, idx:idx + 1], None, op0=ALU.mult
                        )
                        nc.vector.tensor_scalar(
                            wt_dst[:, dk, ch * 1024:(ch + 1) * 1024],
                            r32[:], 1, -1, op0=ALU.min, op1=ALU.max,
                        )
            for ft in range(NFT):
                wtile = wstream.tile([128, 1024], dt.float32, tag="wstream")
                nc.sync.dma_start(wtile[:], wd_ext[l, ft * 128:(ft + 1) * 128, :])
                r32 = wi32.tile([128, 1024], dt.int32, tag="wi32")
                nc.vector.tensor_scalar(
                    r32[:], wtile[:], wsc[:, 3 * l + 2:3 * l + 3], None, op0=ALU.mult
                )
                nc.vector.tensor_scalar(
                    wd_t[:, ft, :], r32[:], 1, -1, op0=ALU.min, op1=ALU.max
                )

            # ---- rms_scale broadcast [128, D] ----
            rs_row = const.tile([1, D], dt.float32, tag="rsrow")
            nc.sync.dma_start(rs_row[:], rs_ext[l:l + 1, :])
            bc_ps = psM.tile([128, D], dt.float32, tag="bcps")
            for h in range(D // 512):
                nc.tensor.matmul(
                    bc_ps[:, h * 512:(h + 1) * 512], ones1[:],
                    rs_row[:, h * 512:(h + 1) * 512], start=True, stop=True,
                )
            scale_bc = bcpool.tile([128, D], dt.float32, tag="bc")
            nc.scalar.activation(scale_bc[:], bc_ps[:], AF.Copy)

            # ---- phase A: per-token-tile row stats (ssq, max|h_pre|) ----
            ssq_all = batch.tile([128, n_tok_tiles], dt.float32, tag="ssq")
            mx_all = batch.tile([128, n_tok_tiles], dt.float32, tag="mx")
            for i in range(n_tok_tiles):
                x1 = xpool.tile([128, D], dt.float32, tag="x1")
                nc.gpsimd.dma_start(x1[:], xsrc[i * 128:(i + 1) * 128, :])
                tt = s4k.tile([128, D], dt.float32, tag="s4k")
                nc.scalar.activation(
                    tt[:], x1[:], AF.Square, accum_out=ssq_all[:, i:i + 1]
                )
                nc.vector.tensor_tensor(tt[:], x1[:], scale_bc[:], op=ALU.mult)
                nc.vector.tensor_reduce(
                    mx_all[:, i:i + 1], tt[:], axis=mybir.AxisListType.X,
                    op=ALU.max, apply_absolute_value=True,
                )
            # batched stats: rstd = 1/sqrt(mean+eps); s = clip(127/(rstd*mx+1e-5))
            ms = batch.tile([128, n_tok_tiles], dt.float32, tag="ms")
            nc.vector.tensor_scalar(ms[:], ssq_all[:], 1.0 / D, 1e-6, op0=ALU.mult, op1=ALU.add)
            rt = batch.tile([128, n_tok_tiles], dt.float32, tag="rt")
            nc.scalar.activation(rt[:], ms[:], AF.Sqrt)
            rstd = batch.tile([128, n_tok_tiles], dt.float32, tag="rstd")
            nc.vector.reciprocal(rstd[:], rt[:])
            # one Newton step: rstd *= 1.5 - 0.5*ms*rstd^2  (fixes the ~7e-6
            # Sqrt-LUT error that quantization tie-flips amplify layer by layer)
            nwt = batch.tile([128, n_tok_tiles], dt.float32, tag="nwt")
            nc.vector.tensor_tensor(nwt[:], rstd[:], rstd[:], op=ALU.mult)
            nc.vector.tensor_tensor(nwt[:], nwt[:], ms[:], op=ALU.mult)
            nc.vector.tensor_scalar(nwt[:], nwt[:], -0.5, 1.5, op0=ALU.mult, op1=ALU.add)
            nc.vector.tensor_tensor(rstd[:], rstd[:], nwt[:], op=ALU.mult)
            maxh = batch.tile([128, n_tok_tiles], dt.float32, tag="maxh")
            nc.vector.tensor_tensor(maxh[:], mx_all[:], rstd[:], op=ALU.mult)
            nc.vector.tensor_scalar(maxh[:], maxh[:], 1e-5, None, op0=ALU.add)
            sr = batch.tile([128, n_tok_tiles], dt.float32, tag="sr")
            nc.vector.reciprocal(sr[:], maxh[:])
            s_all = batch.tile([128, n_tok_tiles], dt.float32, tag="s_all")
            nc.vector.tensor_scalar(s_all[:], sr[:], 127.0, 1e3, op0=ALU.mult, op1=ALU.min)
            nc.vector.tensor_scalar(s_all[:], s_all[:], 1e-3, None, op0=ALU.max)
            c1_all = batch.tile([128, n_tok_tiles], dt.float32, tag="c1")
            nc.vector.tensor_tensor(c1_all[:], s_all[:], rstd[:], op=ALU.mult)
            rs_all = batch.tile([128, n_tok_tiles], dt.float32, tag="rs_all")
            nc.vector.reciprocal(rs_all[:], s_all[:])

            # ---- phase B: per token tile, full GLU ----
            xdst = out_ext if l == n_layers - 1 else dram.tile([tok, D], dt.float32, tag="xbuf")
            for i in range(n_tok_tiles):
                x1 = xpool.tile([128, D], dt.float32, tag="x1")
                nc.gpsimd.dma_start(x1[:], xsrc[i * 128:(i + 1) * 128, :])
                g = gpool.tile([128, F], dt.float32, tag="g")
                t1 = g[:, 0:D]  # scratch alias: consumed before g chunks are written
                nc.vector.tensor_tensor(t1[:], x1[:], scale_bc[:], op=ALU.mult)
                # q = round(c1 * t1) via magic add/sub (exact RNE), out bf16
                nc.scalar.activation(t1[:], t1[:], AF.Identity,
                                     scale=c1_all[:, i:i + 1], bias=mag[:])
                q = qpool.tile([128, D], dt.bfloat16, tag="q")
                nc.scalar.activation(q[:], t1[:], AF.Identity, bias=nmag[:])
                qT = qtpool.tile([128, NDK, 128], dt.bfloat16, tag="qt")
                nc.sync.dma_start_transpose(qT[:], q[:])

                gm8 = sc.tile([128, NFC], dt.float32, tag="gm8")
                for f in range(NFC):
                    u_ps = psA.tile([128, 512], dt.float32, tag="ups")
                    v_ps = psB.tile([128, 512], dt.float32, tag="vps")
                    for dk in range(NDK):
                        nc.tensor.matmul(
                            u_ps[:], qT[:, dk, :], wg_t[:, dk, f * 512:(f + 1) * 512],
                            start=(dk == 0), stop=(dk == NDK - 1),
                        )
                        nc.tensor.matmul(
                            v_ps[:], qT[:, dk, :], wu_t[:, dk, f * 512:(f + 1) * 512],
                            start=(dk == 0), stop=(dk == NDK - 1),
                        )
                    su = silupool.tile([128, 512], dt.float32, tag="silu")
                    nc.scalar.activation(su[:], u_ps[:], AF.Silu, scale=rs_all[:, i:i + 1])
                    nc.vector.tensor_tensor(
                        g[:, f * 512:(f + 1) * 512], su[:], v_ps[:], op=ALU.mult
                    )
                    nc.vector.tensor_reduce(
                        gm8[:, f:f + 1], g[:, f * 512:(f + 1) * 512],
                        axis=mybir.AxisListType.X, op=ALU.max, apply_absolute_value=True,
                    )
                # s2 = clip(127/(max|g|/s + 1e-5)); c2 = s2/s ; rs2 = 1/s2
                gmx = sc.tile([128, 1], dt.float32, tag="gmx")
                nc.vector.tensor_reduce(
                    gmx[:], gm8[:], axis=mybir.AxisListType.X, op=ALU.max,
                    apply_absolute_value=False,
                )
                nc.vector.tensor_tensor(gmx[:], gmx[:], rs_all[:, i:i + 1], op=ALU.mult)
                nc.vector.tensor_scalar(gmx[:], gmx[:], 1e-5, None, op0=ALU.add)
                s2r = sc.tile([128, 1], dt.float32, tag="s2r")
                nc.vector.reciprocal(s2r[:], gmx[:])
                s2 = sc.tile([128, 1], dt.float32, tag="s2")
                nc.vector.tensor_scalar(s2[:], s2r[:], 127.0, 1e3, op0=ALU.mult, op1=ALU.min)
                nc.vector.tensor_scalar(s2[:], s2[:], 1e-3, None, op0=ALU.max)
                c2 = sc.tile([128, 1], dt.float32, tag="c2")
                nc.vector.tensor_tensor(c2[:], s2[:], rs_all[:, i:i + 1], op=ALU.mult)
                rs2 = sc.tile([128, 1], dt.float32, tag="rs2")
                nc.vector.reciprocal(rs2[:], s2[:])
                # gq = round(c2*g) via magic, out bf16
                nc.scalar.activation(g[:], g[:], AF.Identity, scale=c2[:], bias=mag[:])
                gq = gqpool.tile([128, F], dt.bfloat16, tag="gq")
                nc.scalar.activation(gq[:], g[:], AF.Identity, bias=nmag[:])
                gqT = gqtpool.tile([128, NFT, 128], dt.bfloat16, tag="gqt")
                nc.sync.dma_start_transpose(gqT[:], gq[:])

                xnew = s4k.tile([128, D], dt.float32, tag="s4k")
                for dc in range(D // 512):
                    xd_ps = psD.tile([128, 512], dt.float32, tag="xdps")
                    for ft in range(NFT):
                        nc.tensor.matmul(
                            xd_ps[:], gqT[:, ft, :], wd_t[:, ft, dc * 512:(dc + 1) * 512],
                            start=(ft == 0), stop=(ft == NFT - 1),
                        )
                    xdr = xdrpool.tile([128, 512], dt.float32, tag="xdr")
                    nc.scalar.activation(xdr[:], xd_ps[:], AF.Copy, scale=rs2[:])
                    nc.vector.tensor_tensor(
                        xnew[:, dc * 512:(dc + 1) * 512],
                        x1[:, dc * 512:(dc + 1) * 512], xdr[:], op=ALU.add,
                    )
                nc.sync.dma_start(xdst[i * 128:(i + 1) * 128, :], xnew[:])
            xsrc = xdst

    _split_excess_waits(nc)
    return nc


_nc_cache = {}


def _get_nc(key=(NCORES, NTT, L)):
    if key not in _nc_cache:
        _nc_cache[key] = build(*key)
    return _nc_cache[key]


def _make_in_maps(x, rs, wg, wu, wd, n_cores=NCORES):
    gsl = D // n_cores
    dsl = F // n_cores
    in_maps = []
    for c in range(n_cores):
        in_maps.append({
            "x": x[c],
            "rs": rs,
            "wg": wg,
            "wu": wu,
            "wd": wd,
            "slg": np.ascontiguousarray(wg[:, c * gsl:(c + 1) * gsl, :]),
            "slu": np.ascontiguousarray(wu[:, c * gsl:(c + 1) * gsl, :]),
            "sld": np.ascontiguousarray(wd[:, c * dsl:(c + 1) * dsl, :]),
        })
    return in_maps


def kernel(x, rms_scale, W_g, W_u, W_d):
    """Full-input entry point: shard over batch, run 8-core SPMD, gather."""
    x = np.ascontiguousarray(np.asarray(x, dtype=np.float32))
    rs = np.ascontiguousarray(np.asarray(rms_scale, dtype=np.float32))
    wg = np.ascontiguousarray(np.asarray(W_g, dtype=np.float32))
    wu = np.ascontiguousarray(np.asarray(W_u, dtype=np.float32))
    wd = np.ascontiguousarray(np.asarray(W_d, dtype=np.float32))
    B, S, Dx = x.shape
    assert (B, S, Dx) == (NCORES, TOK, D), (B, S, Dx)
    nc = _get_nc()
    in_maps = _make_in_maps(x, rs, wg, wu, wd)
    res = run_bass_kernel_spmd(nc, in_maps, list(range(NCORES)))
    return np.stack([res.results[c]["out"] for c in range(NCORES)], axis=0)


# revision 13
# speedup vs baseline: 1.6674x; 1.0448x over previous
"""Trainium2 Bass kernel for nn_CascadeTransformerMM (4-layer ternary-GLU cascade).

Math (per layer, per token row):
  h   = rms_scale * x * rsqrt(mean(x^2) + 1e-6)
  s   = clip(127/(max|h| + 1e-5), 1e-3, 1e3);  q = round(s*h)      (ints in [-127,127])
  Wt  = clip(round(W * 127/(max|W| + 1e-5)), -1, 1)                 (ternary {-1,0,1})
  u   = (q @ Wg_t)/s ; v = (q @ Wu_t)/s ; g = silu(u)*v
  s2  = clip(127/(max|g| + 1e-5), 1e-3, 1e3); gq = round(s2*g)
  x  += (gq @ Wd_t)/s2

Distribution: pure data-parallel over the batch dim (8 batches -> 8 cores),
weights replicated per core. Per-matrix |W|max is computed cooperatively:
each core reduces a 1/8 row-slice, then a tiny AllReduce(max) shares the 12
scalars. All matmuls run on the PE array with bf16 activations (integers
<= 127, exact) x fp8 ternary weights (exact), fp32 PSUM accumulation -> the
heavy compute is bit-exact integer arithmetic.

Layout: token-major activations [128 tok, D]; q/gq transposed to feature-major
via single xbar DMA-transposes; weights ternarized on the fly into SBUF-resident
fp8 tiles (int32-convert round + chained min/max clip, both exact)."""

import os
import sys

for _p in ("/opt/trn_rl_repo", "/root/.axon_site/_ro/trn_rl_repo"):
    if os.path.isdir(_p) and _p not in sys.path:
        sys.path.insert(0, _p)

import numpy as np
from contextlib import ExitStack

import concourse.bass as bass
import concourse.mybir as mybir
import concourse.tile as tile
from concourse.bass_utils import run_bass_kernel_spmd

dt = mybir.dt
AF = mybir.ActivationFunctionType
ALU = mybir.AluOpType

MAGIC = float(1.5 * 2**23)  # fp32 round-to-nearest-even magic constant
D = 1024
F = 4096
L = 4
NCORES = 8
TOK = 1024  # tokens per core (one batch of S=1024)

NDK = D // 128   # 8 contraction tiles for up-proj
NFT = F // 128   # 32 contraction tiles for down-proj
NFC = F // 512   # 8 free-dim chunks for up-proj
NTT = TOK // 128  # 8 token tiles


def _split_excess_waits(nc, max_waits: int = 1) -> int:
    """walrus in this container rejects >1 sync-wait per instruction; split
    extras into standalone event-semaphore waits on the same engine (same-
    engine program order makes this semantically identical)."""
    n = 0
    for func in nc.m.functions:
        for block in func.blocks:
            changed = False
            out = []
            for inst in block.instructions:
                si = getattr(inst, "sync_info", None)
                if si is not None and si.on_wait and len(si.on_wait) > max_waits:
                    waits = list(si.on_wait)
                    for j, w in enumerate(waits[max_waits:]):
                        out.append(
                            mybir.InstEventSemaphore(
                                name=f"{inst.name}-xw{j}",
                                engine=inst.engine,
                                ins=[],
                                outs=[],
                                sync_info=mybir.SyncInfo(on_wait=[w], on_update=[]),
                            )
                        )
                        n += 1
                    inst.sync_info = mybir.SyncInfo(
                        on_wait=waits[:max_waits], on_update=list(si.on_update)
                    )
                    changed = True
                out.append(inst)
            if changed:
                block.instructions = out
    return n


def build(n_cores: int = NCORES, n_tok_tiles: int = NTT, n_layers: int = L) -> bass.Bass:
    nc = bass.Bass(num_devices=n_cores)
    tok = n_tok_tiles * 128

    x_ext = nc.declare_dram_parameter("x", [tok, D], dt.float32, isOutput=False)
    rs_ext = nc.declare_dram_parameter("rs", [n_layers, D], dt.float32, isOutput=False)
    # wg/wu repacked host-side to [L, F//1024, NDK, 128, 1024] so every
    # [128, 1024] weight tile is one contiguous 512 KB DMA; wd's row-slabs
    # are naturally contiguous.
    wg_ext = nc.declare_dram_parameter("wg", [n_layers, F // 1024, NDK, 128, 1024], dt.float32, isOutput=False)
    wu_ext = nc.declare_dram_parameter("wu", [n_layers, F // 1024, NDK, 128, 1024], dt.float32, isOutput=False)
    wd_ext = nc.declare_dram_parameter("wd", [n_layers, F, D], dt.float32, isOutput=False)
    # per-core row-slices of each matrix for the cooperative |W|max,
    # as [L, nunit, 128, 1024] contiguous units
    nun = (D // n_cores) * F // (128 * 1024)
    slg_ext = nc.declare_dram_parameter("slg", [n_layers, nun, 128, 1024], dt.float32, isOutput=False)
    slu_ext = nc.declare_dram_parameter("slu", [n_layers, nun, 128, 1024], dt.float32, isOutput=False)
    sld_ext = nc.declare_dram_parameter("sld", [n_layers, nun, 128, 1024], dt.float32, isOutput=False)
    out_ext = nc.declare_dram_parameter("out", [tok, D], dt.float32, isOutput=True)

    nmat = 3 * n_layers
    mx_loc = nc.dram_tensor("mx_loc", [1, 16], dt.float32)
    mx_glob = nc.dram_tensor("mx_glob", [1, 16], dt.float32)

    with tile.TileContext(nc) as tc, ExitStack() as ctx:
        P = ctx.enter_context
        wpool = P(tc.tile_pool(name="wts", bufs=3))
        wstream = P(tc.tile_pool(name="wstream", bufs=2))
        wi32 = P(tc.tile_pool(name="wi32", bufs=2))
        xpool = P(tc.tile_pool(name="x1", bufs=2))
        s4k = P(tc.tile_pool(name="s4k", bufs=2))       # t1_tmp / xnew scratch
        qpool = P(tc.tile_pool(name="q", bufs=1))
        qtpool = P(tc.tile_pool(name="qt", bufs=2))
        silupool = P(tc.tile_pool(name="silu", bufs=1))
        gpool = P(tc.tile_pool(name="g", bufs=2))
        gqpool = P(tc.tile_pool(name="gq", bufs=1))
        gqtpool = P(tc.tile_pool(name="gqt", bufs=2))
        bcpool = P(tc.tile_pool(name="bc", bufs=1))
        batch = P(tc.tile_pool(name="batch", bufs=2))   # [128, ntt] per-layer stats
        sc = P(tc.tile_pool(name="sc", bufs=4))         # [128, small] scalars
        xdrpool = P(tc.tile_pool(name="xdr", bufs=2))
        const = P(tc.tile_pool(name="const", bufs=1))
        dram = P(tc.tile_pool(name="dram", bufs=2, space="DRAM"))
        psA = P(tc.tile_pool(name="psA", bufs=2, space="PSUM"))
        psB = P(tc.tile_pool(name="psB", bufs=2, space="PSUM"))
        psD = P(tc.tile_pool(name="psD", bufs=2, space="PSUM"))
        psM = P(tc.tile_pool(name="psM", bufs=1, space="PSUM"))

        # ---------- constants ----------
        ones1 = const.tile([1, 128], dt.float32, tag="ones1")
        nc.gpsimd.memset(ones1[:], 1.0)
        mag = const.tile([128, 1], dt.float32, tag="mag")
        nc.gpsimd.memset(mag[:], MAGIC)
        nmag = const.tile([128, 1], dt.float32, tag="nmag")
        nc.gpsimd.memset(nmag[:], -MAGIC)
        c127 = const.tile([128, 16], dt.float32, tag="c127")
        nc.gpsimd.memset(c127[:], 127.0)

        # ---------- cooperative per-matrix |W|max ----------
        wmax_cols = const.tile([128, 16], dt.float32, tag="wmaxc")
        nc.gpsimd.memset(wmax_cols[:], 0.0)
        for l in range(n_layers):
            for mi, ext in enumerate((slg_ext, slu_ext, sld_ext)):
                idx = 3 * l + mi
                part = sc.tile([128, nun], dt.float32, tag="wmaxpart")
                for un in range(nun):
                    wt = wstream.tile([128, 1024], dt.float32, tag="wstream")
                    nc.sync.dma_start(wt[:], ext[l, un])
                    nc.vector.tensor_reduce(
                        part[:, un:un + 1], wt[:], axis=mybir.AxisListType.X,
                        op=ALU.max, apply_absolute_value=True,
                    )
                nc.vector.tensor_reduce(
                    wmax_cols[:, idx:idx + 1], part[:, 0:nun],
                    axis=mybir.AxisListType.X, op=ALU.max,
                    apply_absolute_value=False,
                )
        # partition-dim reduce 128 -> 1 (gpsimd), pad row to 16, AllReduce(max)
        mrow = const.tile([1, 16], dt.float32, tag="mrow")
        nc.gpsimd.memset(mrow[:], 0.0)
        nc.gpsimd.tensor_reduce(
            mrow[:, 0:nmat], wmax_cols[:, 0:nmat], axis=mybir.AxisListType.C, op=ALU.max
        )
        nc.sync.dma_start(mx_loc[:], mrow[:])
        nc.gpsimd.collective_compute(
            "AllReduce",
            ALU.max,
            replica_groups=[list(range(n_cores))],
            ins=[mx_loc[:].opt()],
            outs=[mx_glob[:].opt()],
        )
        grow = const.tile([1, 16], dt.float32, tag="grow")
        nc.sync.dma_start(grow[:], mx_glob[:])
        # broadcast to all partitions via PE outer product; wsc = 127/(m+1e-5)
        msc_ps = psM.tile([128, D], dt.float32, tag="bcps")
        nc.tensor.matmul(msc_ps[:, 0:16], ones1[:], grow[:], start=True, stop=True)
        msc = const.tile([128, 16], dt.float32, tag="msc")
        nc.vector.tensor_scalar(msc[:], msc_ps[:, 0:16], 1e-5, None, op0=ALU.add)
        mrec = const.tile([128, 16], dt.float32, tag="mrec")
        nc.vector.reciprocal(mrec[:], msc[:])
        wsc = const.tile([128, 16], dt.float32, tag="wsc")
        nc.vector.tensor_scalar(wsc[:], mrec[:], 127.0, None, op0=ALU.mult)

        # ---------- layers ----------
        xsrc = x_ext
        for l in range(n_layers):
            # ---- ternarize weights into SBUF-resident fp8 tiles ----
            wg_t = wpool.tile([128, NDK, F], dt.float8e4, tag="wts")
            wu_t = wpool.tile([128, NDK, F], dt.float8e4, tag="wts")
            wd_t = wpool.tile([128, NFT, D], dt.float8e4, tag="wts")
            for ch in range(F // 1024):  # chunk-outer: early F-chunks ready first
                for dk in range(NDK):
                    for ext, wt_dst, idx in ((wg_ext, wg_t, 3 * l), (wu_ext, wu_t, 3 * l + 1)):
                        wtile = wstream.tile([128, 1024], dt.float32, tag="wstream")
                        nc.sync.dma_start(wtile[:], ext[l, ch, dk])
                        r32 = wi32.tile([128, 1024], dt.int32, tag="wi32")
                        nc.vector.tensor_scalar(
                            r32[:], wtile[:], wsc[:, idx:idx + 1], None, op0=ALU.mult
                        )
                        nc.vector.tensor_scalar(
                            wt_dst[:, dk, ch * 1024:(ch + 1) * 1024],
                            r32[:], 1, -1, op0=ALU.min, op1=ALU.max,
                        )
            for ft in range(NFT):
                wtile = wstream.tile([128, 1024], dt.float32, tag="wstream")
                nc.sync.dma_start(wtile[:], wd_ext[l, ft * 128:(ft + 1) * 128, :])
                r32 = wi32.tile([128, 1024], dt.int32, tag="wi32")
                nc.vector.tensor_scalar(
                    r32[:], wtile[:], wsc[:, 3 * l + 2:3 * l + 3], None, op0=ALU.mult
                )
                nc.vector.tensor_scalar(
                    wd_t[:, ft, :], r32[:], 1, -1, op0=ALU.min, op1=ALU.max
                )

            # ---- rms_scale broadcast [128, D] ----
            rs_row = const.tile([1, D], dt.float32, tag="rsrow")
            nc.sync.dma_start(rs_row[:], rs_ext[l:l + 1, :])
            bc_ps = psM.tile([128, D], dt.float32, tag="bcps")
            for h in range(D // 512):
                nc.tensor.matmul(
                    bc_ps[:, h * 512:(h + 1) * 512], ones1[:],
                    rs_row[:, h * 512:(h + 1) * 512], start=True, stop=True,
                )
            scale_bc = bcpool.tile([128, D], dt.float32, tag="bc")
            nc.scalar.activation(scale_bc[:], bc_ps[:], AF.Copy)

            # ---- phase A: per-token-tile row stats (ssq, max|h_pre|) ----
            ssq_all = batch.tile([128, n_tok_tiles], dt.float32, tag="ssq")
            mx_all = batch.tile([128, n_tok_tiles], dt.float32, tag="mx")
            for i in range(n_tok_tiles):
                x1 = xpool.tile([128, D], dt.float32, tag="x1")
                nc.gpsimd.dma_start(x1[:], xsrc[i * 128:(i + 1) * 128, :])
                tt = s4k.tile([128, D], dt.float32, tag="s4k")
                nc.scalar.activation(
                    tt[:], x1[:], AF.Square, accum_out=ssq_all[:, i:i + 1]
                )
                nc.vector.tensor_tensor(tt[:], x1[:], scale_bc[:], op=ALU.mult)
                nc.vector.tensor_reduce(
                    mx_all[:, i:i + 1], tt[:], axis=mybir.AxisListType.X,
                    op=ALU.max, apply_absolute_value=True,
                )
            # batched stats: rstd = 1/sqrt(mean+eps); s = clip(127/(rstd*mx+1e-5))
            ms = batch.tile([128, n_tok_tiles], dt.float32, tag="ms")
            nc.vector.tensor_scalar(ms[:], ssq_all[:], 1.0 / D, 1e-6, op0=ALU.mult, op1=ALU.add)
            rt = batch.tile([128, n_tok_tiles], dt.float32, tag="rt")
            nc.scalar.activation(rt[:], ms[:], AF.Sqrt)
            rstd = batch.tile([128, n_tok_tiles], dt.float32, tag="rstd")
            nc.vector.reciprocal(rstd[:], rt[:])
            # one Newton step: rstd *= 1.5 - 0.5*ms*rstd^2  (fixes the ~7e-6
            # Sqrt-LUT error that quantization tie-flips amplify layer by layer)
            nwt = batch.tile([128, n_tok_tiles], dt.float32, tag="nwt")
            nc.vector.tensor_tensor(nwt[:], rstd[:], rstd[:], op=ALU.mult)
            nc.vector.tensor_tensor(nwt[:], nwt[:], ms[:], op=ALU.mult)
            nc.vector.tensor_scalar(nwt[:], nwt[:], -0.5, 1.5, op0=ALU.mult, op1=ALU.add)
            nc.vector.tensor_tensor(rstd[:], rstd[:], nwt[:], op=ALU.mult)
            maxh = batch.tile([128, n_tok_tiles], dt.float32, tag="maxh")
            nc.vector.tensor_tensor(maxh[:], mx_all[:], rstd[:], op=ALU.mult)
            nc.vector.tensor_scalar(maxh[:], maxh[:], 1e-5, None, op0=ALU.add)
            sr = batch.tile([128, n_tok_tiles], dt.float32, tag="sr")
            nc.vector.reciprocal(sr[:], maxh[:])
            s_all = batch.tile([128, n_tok_tiles], dt.float32, tag="s_all")
            nc.vector.tensor_scalar(s_all[:], sr[:], 127.0, 1e3, op0=ALU.mult, op1=ALU.min)
            nc.vector.tensor_scalar(s_all[:], s_all[:], 1e-3, None, op0=ALU.max)
            c1_all = batch.tile([128, n_tok_tiles], dt.float32, tag="c1")
            nc.vector.tensor_tensor(c1_all[:], s_all[:], rstd[:], op=ALU.mult)
            rs_all = batch.tile([128, n_tok_tiles], dt.float32, tag="rs_all")
            nc.vector.reciprocal(rs_all[:], s_all[:])

            # ---- phase B: per token tile, full GLU ----
            xdst = out_ext if l == n_layers - 1 else dram.tile([tok, D], dt.float32, tag="xbuf")
            for i in range(n_tok_tiles):
                x1 = xpool.tile([128, D], dt.float32, tag="x1")
                nc.gpsimd.dma_start(x1[:], xsrc[i * 128:(i + 1) * 128, :])
                g = gpool.tile([128, F], dt.float32, tag="g")
                t1 = g[:, 0:D]  # scratch alias: consumed before g chunks are written
                nc.vector.tensor_tensor(t1[:], x1[:], scale_bc[:], op=ALU.mult)
                # q = round(c1 * t1) via magic add/sub (exact RNE), out bf16
                nc.scalar.activation(t1[:], t1[:], AF.Identity,
                                     scale=c1_all[:, i:i + 1], bias=mag[:])
                q = qpool.tile([128, D], dt.bfloat16, tag="q")
                nc.scalar.activation(q[:], t1[:], AF.Identity, bias=nmag[:])
                qT = qtpool.tile([128, NDK, 128], dt.bfloat16, tag="qt")
                nc.sync.dma_start_transpose(qT[:], q[:])

                gm8 = sc.tile([128, NFC], dt.float32, tag="gm8")
                for f in range(NFC):
                    u_ps = psA.tile([128, 512], dt.float32, tag="ups")
                    v_ps = psB.tile([128, 512], dt.float32, tag="vps")
                    for dk in range(NDK):
                        nc.tensor.matmul(
                            u_ps[:], qT[:, dk, :], wg_t[:, dk, f * 512:(f + 1) * 512],
                            start=(dk == 0), stop=(dk == NDK - 1),
                        )
                        nc.tensor.matmul(
                            v_ps[:], qT[:, dk, :], wu_t[:, dk, f * 512:(f + 1) * 512],
                            start=(dk == 0), stop=(dk == NDK - 1),
                        )
                    su = silupool.tile([128, 512], dt.float32, tag="silu")
                    nc.scalar.activation(su[:], u_ps[:], AF.Silu, scale=rs_all[:, i:i + 1])
                    nc.vector.tensor_tensor(
                        g[:, f * 512:(f + 1) * 512], su[:], v_ps[:], op=ALU.mult
                    )
                    nc.vector.tensor_reduce(
                        gm8[:, f:f + 1], g[:, f * 512:(f + 1) * 512],
                        axis=mybir.AxisListType.X, op=ALU.max, apply_absolute_value=True,
                    )
                # s2 = clip(127/(max|g|/s + 1e-5)); c2 = s2/s ; rs2 = 1/s2
                gmx = sc.tile([128, 1], dt.float32, tag="gmx")
                nc.vector.tensor_reduce(
                    gmx[:], gm8[:], axis=mybir.AxisListType.X, op=ALU.max,
                    apply_absolute_value=False,
                )
                nc.vector.tensor_tensor(gmx[:], gmx[:], rs_all[:, i:i + 1], op=ALU.mult)
                nc.vector.tensor_scalar(gmx[:], gmx[:], 1e-5, None, op0=ALU.add)
                s2r = sc.tile([128, 1], dt.float32, tag="s2r")
                nc.vector.reciprocal(s2r[:], gmx[:])
                s2 = sc.tile([128, 1], dt.float32, tag="s2")
                nc.vector.tensor_scalar(s2[:], s2r[:], 127.0, 1e3, op0=ALU.mult, op1=ALU.min)
                nc.vector.tensor_scalar(s2[:], s2[:], 1e-3, None, op0=ALU.max)
                c2 = sc.tile([128, 1], dt.float32, tag="c2")
                nc.vector.tensor_tensor(c2[:], s2[:], rs_all[:, i:i + 1], op=ALU.mult)
                rs2 = sc.tile([128, 1], dt.float32, tag="rs2")
                nc.vector.reciprocal(rs2[:], s2[:])
                # gq = round(c2*g) via magic, out bf16
                nc.scalar.activation(g[:], g[:], AF.Identity, scale=c2[:], bias=mag[:])
                gq = gqpool.tile([128, F], dt.bfloat16, tag="gq")
                nc.scalar.activation(gq[:], g[:], AF.Identity, bias=nmag[:])
                gqT = gqtpool.tile([128, NFT, 128], dt.bfloat16, tag="gqt")
                nc.sync.dma_start_transpose(gqT[:], gq[:])

                xnew = s4k.tile([128, D], dt.float32, tag="s4k")
                xd_ps0 = psD.tile([128, 512], dt.float32, tag="xdps")
                xd_ps1 = psD.tile([128, 512], dt.float32, tag="xdps")
                for ft in range(NFT):
                    nc.tensor.matmul(
                        xd_ps0[:], gqT[:, ft, :], wd_t[:, ft, 0:512],
                        start=(ft == 0), stop=(ft == NFT - 1),
                    )
                    nc.tensor.matmul(
                        xd_ps1[:], gqT[:, ft, :], wd_t[:, ft, 512:1024],
                        start=(ft == 0), stop=(ft == NFT - 1),
                    )
                for dc, xd_ps in ((0, xd_ps0), (1, xd_ps1)):
                    xdr = xdrpool.tile([128, 512], dt.float32, tag="xdr")
                    nc.scalar.activation(xdr[:], xd_ps[:], AF.Copy, scale=rs2[:])
                    nc.vector.tensor_tensor(
                        xnew[:, dc * 512:(dc + 1) * 512],
                        x1[:, dc * 512:(dc + 1) * 512], xdr[:], op=ALU.add,
                    )
                nc.sync.dma_start(xdst[i * 128:(i + 1) * 128, :], xnew[:])
            xsrc = xdst

    _split_excess_waits(nc)
    return nc


_nc_cache = {}


def _get_nc(key=(NCORES, NTT, L)):
    if key not in _nc_cache:
        _nc_cache[key] = build(*key)
    return _nc_cache[key]


def _repack(w, n_layers):
    # [L, D, F] -> [L, F//1024, D//128, 128, 1024] so each (ch, dk) tile is
    # contiguous; dk == core index for the per-core max slices.
    return np.ascontiguousarray(
        w.reshape(n_layers, D // 128, 128, F // 1024, 1024).transpose(0, 3, 1, 2, 4)
    )


def _make_in_maps(x, rs, wg, wu, wd, n_cores=NCORES):
    n_layers = rs.shape[0]
    wg_r = _repack(wg, n_layers)
    wu_r = _repack(wu, n_layers)
    nsl = NDK // n_cores  # dk-slabs per core for slg/slu
    dsl = F // n_cores
    in_maps = []
    for c in range(n_cores):
        slg = wg_r[:, :, c * nsl:(c + 1) * nsl].reshape(n_layers, -1, 128, 1024)
        slu = wu_r[:, :, c * nsl:(c + 1) * nsl].reshape(n_layers, -1, 128, 1024)
        sld = wd[:, c * dsl:(c + 1) * dsl, :].reshape(n_layers, -1, 128, 1024)
        in_maps.append({
            "x": x[c],
            "rs": rs,
            "wg": wg_r,
            "wu": wu_r,
            "wd": wd,
            "slg": np.ascontiguousarray(slg),
            "slu": np.ascontiguousarray(slu),
            "sld": np.ascontiguousarray(sld),
        })
    return in_maps


def kernel(x, rms_scale, W_g, W_u, W_d):
    """Full-input entry point: shard over batch, run 8-core SPMD, gather."""
    x = np.ascontiguousarray(np.asarray(x, dtype=np.float32))
    rs = np.ascontiguousarray(np.asarray(rms_scale, dtype=np.float32))
    wg = np.ascontiguousarray(np.asarray(W_g, dtype=np.float32))
    wu = np.ascontiguousarray(np.asarray(W_u, dtype=np.float32))
    wd = np.ascontiguousarray(np.asarray(W_d, dtype=np.float32))
    B, S, Dx = x.shape
    assert (B, S, Dx) == (NCORES, TOK, D), (B, S, Dx)
    nc = _get_nc()
    in_maps = _make_in_maps(x, rs, wg, wu, wd)
    res = run_bass_kernel_spmd(nc, in_maps, list(range(NCORES)))
    return np.stack([res.results[c]["out"] for c in range(NCORES)], axis=0)
